# revision 1
# baseline (speedup 1.0000x reference)
"""CostVolumeLayer Trainium2 kernel.

Computes the local cost volume: for search_range R=4,
  out[b, di*9+dj, i, j] = sum_c src[b,c,i,j] * tgt_zp[b,c,i-2R+di, j-2R+dj]
(tgt zero-padded outside its bounds; the window is OFF-CENTER, covering
tgt rows i-8..i and cols j-8..j — faithful to the torch reference, whose
window indices index the zero-padded tensor directly and whose negative
indices wrap into the zero pad).

Strategy (8 NeuronCores, SPMD):
  - Shard: core c -> batch b = c//2, row-half r0 = 32*(c%2). Each core gets
    src shard [C=128, 32, 128] and a zero-padded tgt halo shard
    [C=128, 40, 136] (host pre-pads; halo = R rows/cols each side).
  - Device: for each 8x16 pixel block of the shard, one TensorE matmul
    lhsT = src block [K=C=128, M=128 pixels], rhs = tgt window
    [K=128, N=16x24=384] -> PSUM Gram [128, 384]; DVE copies PSUM->SBUF;
    DMA to DRAM.
  - Host: zero-FLOP banded-diagonal gather from the Gram blocks into the
    [B, 81, H, W] output (the 81 needed entries per pixel live at
    n = (mi+di)*24 + (mj+dj), a per-partition-skewed pattern that engine
    access patterns cannot express on-chip).
"""

import numpy as np

R = 4
D = 2 * R + 1          # 9
B, C, H, W = 4, 128, 64, 128
NCORES = 8
HS = H // 2            # 32 rows per core shard
TH = HS + 2 * R        # 40 padded tgt rows per shard
TW = W + 2 * R         # 136 padded tgt cols
BI, BJ = 8, 16         # pixel block: 8 rows x 16 cols = 128 = M
NBI, NBJ = HS // BI, W // BJ   # 4 x 8 = 32 blocks per core
WIN_I, WIN_J = BI + 2 * R, BJ + 2 * R  # 16 x 24 window
NW = WIN_I * WIN_J     # 384 streamed columns per block
NBLK = NBI * NBJ
GRP = 4                # blocks per output DMA group
NGRP = NBLK // GRP     # 8 output DMAs
# Band dump: partition group mi only needs Gram cols 24*mi..24*mi+215, so
# partitions 0..63 (groups 0-3) keep cols 0..287 and partitions 64..127
# (groups 4-7) keep cols 96..383. The PSUM->SBUF copies select the band
# (engines accept partition subranges); the output DMA stays a plain
# full-partition transfer. 25% fewer dump bytes, numerically exact.
BANDW = NW - 4 * WIN_J  # 288
BANDO = 4 * WIN_J       # 96, column offset of the upper-half band
DUMP_FP16 = False      # dump Gram blocks as fp16 (halves output DMA bytes);
                       # adds ~5e-4 absmax-relative error from output rounding
SPLIT_BF16 = False     # compute src.T@tgt as 3 accumulating bf16 matmuls on
                       # hi/lo split operands (3 cyc/row vs fp32's 4, FWL
                       # weight loads); ~1e-5 absmax-relative error. Measured
                       # slower than plain fp32 (52us vs 43us): the 3x
                       # instruction count outweighs the per-row savings.

_compiled = None


def _build_bass():
    import concourse.mybir as mybir
    from concourse import bacc
    from concourse.tile import TileContext
    from concourse.tile_rust import add_dep_helper

    f32 = mybir.dt.float32
    nc = bacc.Bacc()
    # single combined input: [C, HS*W (block-reordered src) ++ TH*TW (padded
    # tgt)]; with SPLIT_BF16 the free dim doubles: [hi || lo] halves in bf16.
    E = HS * W + TH * TW
    if SPLIT_BF16:
        in_dt = mybir.dt.bfloat16
        inp = nc.dram_tensor("inp", [C, 2 * E], in_dt, kind="ExternalInput")
    else:
        in_dt = f32
        inp = nc.dram_tensor("inp", [C, E], in_dt, kind="ExternalInput")
    # Gram dump, group-major: NGRP groups of GRP blocks staged contiguously
    # so each group leaves as one plain [128, GRP*NW] DMA (one sem wait each;
    # the HWDGE pseudo-DMA instruction can encode only one sync wait).
    dump_dt = mybir.dt.float16 if DUMP_FP16 else f32
    gout = nc.dram_tensor("gout", [NGRP, 128, GRP * BANDW], dump_dt,
                          kind="ExternalOutput")
    gout_ap = gout.ap()

    with TileContext(nc) as tc:
        with (
            tc.tile_pool(name="inp", bufs=1) as inp_pool,
            tc.tile_pool(name="g", bufs=NGRP) as gpool,
            tc.tile_pool(name="psum", bufs=7, space="PSUM") as psum_pool,
            tc.tile_pool(name="warmpsum", bufs=1, space="PSUM") as warm_pool,
        ):
            # src arrives block-reordered from the host: [C, blk, 128 pixels]
            # so each block's weights are one contiguous free dim (the matmul
            # stationary operand allows only one free dimension).
            # Inputs are loaded in row-chunks so the first block-row's
            # matmuls start after ~1.4MB instead of after the full 4.8MB
            # (Tile's dependency tracking is AP-range-aware).
            nhalf = 2 if SPLIT_BF16 else 1
            a = inp_pool.tile([C, nhalf * E], in_dt)
            # [C, half, E] views over both SBUF tile and DRAM input: one
            # chunk DMA moves the hi and lo copies of a region together.
            av = a.rearrange("c (h e) -> c h e", h=nhalf)
            iv = inp.ap().rearrange("c (h e) -> c h e", h=nhalf)

            def s_half(h):
                return av[:, h, :HS * W]

            def t_half(h):
                return av[:, h, HS * W:].rearrange("c (i j) -> c i j", j=TW)

            # PE warm-up: dummy matmuls during the input-DMA wait flip the
            # HAM clock gate to 8/8 before the real matmuls start.
            warm = inp_pool.tile([128, 128], in_dt)
            nc.vector.memset(warm, 0.0)
            wps = warm_pool.tile([1, 128], f32)
            for _ in range(32 if SPLIT_BF16 else 12):
                nc.tensor.matmul(wps, warm[:, :1], warm, start=True, stop=True)

            # Chunked input load. The SDMA engines round-robin across queues,
            # so concurrent chunk DMAs all finish together; chain the
            # non-critical chunks behind the three the first block-row needs
            # so those get full bandwidth.
            SRCC = NBJ * 128            # src chunk: one block-row = 1024 elems
            TGTC = 8 * TW               # tgt chunk: 8 rows
            def src_chunk(i):
                return nc.sync.dma_start(
                    out=av[:, :, i * SRCC:(i + 1) * SRCC],
                    in_=iv[:, :, i * SRCC:(i + 1) * SRCC])
            def tgt_chunk(i):
                o = HS * W + i * TGTC
                return nc.sync.dma_start(out=av[:, :, o:o + TGTC],
                                         in_=iv[:, :, o:o + TGTC])
            first = [src_chunk(0), tgt_chunk(0), tgt_chunk(1),
                     src_chunk(1), tgt_chunk(2)]
            rest = [src_chunk(2), tgt_chunk(3),
                    src_chunk(3), tgt_chunk(4)]
            for r in rest:
                for f in first:
                    add_dep_helper(r.ins, f.ins,
                                   reason="critical input chunks drain first")

            for grp in range(NGRP):
                stage = gpool.tile([128, GRP * BANDW], dump_dt)
                for k in range(GRP):
                    blk = grp * GRP + k
                    bi, bj = divmod(blk, NBJ)
                    ps = psum_pool.tile([128, NW], f32)
                    if SPLIT_BF16:
                        # x*y ~= xh*yh + xh*yl + xl*yh (lo*lo term ~2^-18)
                        terms = [(0, 0, True, False), (0, 1, False, False),
                                 (1, 0, False, True)]
                    else:
                        terms = [(0, 0, True, True)]
                    for sh, th, st, sp in terms:
                        lhsT = s_half(sh)[:, blk * 128:(blk + 1) * 128]
                        rhs = t_half(th)[:, bi * BI: bi * BI + WIN_I,
                                         bj * BJ: bj * BJ + WIN_J]
                        nc.tensor.matmul(ps, lhsT, rhs, start=st, stop=sp)
                    # band-select copies: lower partition half on DVE, upper
                    # half (shifted by BANDO) on ACT - balanced in parallel
                    dst = stage[:, k * BANDW:(k + 1) * BANDW]
                    nc.vector.tensor_copy(dst[0:64], ps[0:64, 0:BANDW])
                    nc.scalar.copy(dst[64:128], ps[64:128, BANDO:BANDO + BANDW])
                nc.sync.dma_start(out=gout_ap[grp], in_=stage)
    nc.finalize()
    return nc


def _get_compiled():
    global _compiled
    if _compiled is None:
        _compiled = _build_bass()
    return _compiled


def _shard_inputs(src, tgt):
    """Build per-core input maps (host-side shard + zero-pad)."""
    in_maps = []
    for c in range(NCORES):
        b = c // 2
        r0 = HS * (c % 2)
        # block-reorder: [C, NBI, BI, NBJ, BJ] -> [C, (NBI NBJ), (BI BJ)]
        s = np.ascontiguousarray(
            src[b, :, r0:r0 + HS, :]
            .reshape(C, NBI, BI, NBJ, BJ)
            .transpose(0, 1, 3, 2, 4)
        ).reshape(C, HS * W)
        tp = np.zeros((C, TH, TW), dtype=np.float32)
        # The reference's window for output pixel (i, j) covers tgt rows
        # i-2R..i and cols j-2R..j (off-center, faithful to the torch quirk:
        # window indices index the PADDED tensor directly, so padded index
        # i-R+di = tgt row i-2R+di). Device pairs src local row il with
        # shard-padded row il+di, so shard row q holds tgt row r0+q-2R;
        # shard col x holds tgt col x-2R.
        lo = r0 - 2 * R
        hi = r0 + HS
        clo = max(lo, 0)
        tp[:, clo - lo: clo - lo + (hi - clo), 2 * R: 2 * R + W] = tgt[b, :, clo:hi, :]
        inp = np.concatenate([s, tp.reshape(C, TH * TW)], axis=1)
        if SPLIT_BF16:
            import ml_dtypes

            bf16 = ml_dtypes.bfloat16
            ihi = inp.astype(bf16)
            ilo = (inp - ihi.astype(np.float32)).astype(bf16)
            inp = np.concatenate([ihi, ilo], axis=1)
        in_maps.append({"inp": np.ascontiguousarray(inp)})
    return in_maps


# host-side gather indices: out[k=(di,dj)] at pixel (mi,mj) of a block sits at
# Gram column n = (mi+di)*WIN_J + (mj+dj); the device band dump stores
# columns shifted by BANDO for partition groups mi >= 4.
_mi = np.arange(BI)[:, None, None, None]
_mj = np.arange(BJ)[None, :, None, None]
_di = np.arange(D)[None, None, :, None]
_dj = np.arange(D)[None, None, None, :]
_NIDX = ((_mi + _di) * WIN_J + (_mj + _dj)
         - BANDO * (_mi >= 4)).reshape(BI, BJ, D * D)  # [8,16,81]


def _unshard_output(results):
    out = np.empty((B, D * D, H, W), dtype=np.float32)
    for c in range(NCORES):
        b = c // 2
        r0 = HS * (c % 2)
        g = (results[c]["gout"]
             .astype(np.float32)
             .reshape(NGRP, 128, GRP, BANDW)
             .transpose(0, 2, 1, 3)
             .reshape(NBI, NBJ, BI, BJ, BANDW))
        # gather: v[bi,bj,mi,mj,k] = g[bi,bj,mi,mj,_NIDX[mi,mj,k]]
        v = np.take_along_axis(g, _NIDX[None, None], axis=-1)
        # -> out[b, k, r0+bi*8+mi, bj*16+mj]
        v = v.transpose(4, 0, 2, 1, 3)  # [81, NBI, BI, NBJ, BJ]
        out[b, :, r0:r0 + HS, :] = v.reshape(D * D, HS, W)
    return out


def kernel(src, tgt):
    from concourse.bass_utils import run_bass_kernel_spmd

    src = np.asarray(src, dtype=np.float32)
    tgt = np.asarray(tgt, dtype=np.float32)
    nc = _get_compiled()
    in_maps = _shard_inputs(src, tgt)
    res = run_bass_kernel_spmd(nc, in_maps, core_ids=list(range(NCORES)))
    return _unshard_output(res.results)



# revision 4
# speedup vs baseline: 1.2297x; 1.2297x over previous
"""CostVolumeLayer Trainium2 kernel.

Computes the local cost volume: for search_range R=4,
  out[b, di*9+dj, i, j] = sum_c src[b,c,i,j] * tgt_zp[b,c,i-2R+di, j-2R+dj]
(tgt zero-padded outside its bounds; the window is OFF-CENTER, covering
tgt rows i-8..i and cols j-8..j — faithful to the torch reference, whose
window indices index the zero-padded tensor directly and whose negative
indices wrap into the zero pad).

Strategy (8 NeuronCores, SPMD):
  - Shard: core c -> batch b = c//2, row-half r0 = 32*(c%2). Each core gets
    src shard [C=128, 32, 128] and a zero-padded tgt halo shard
    [C=128, 40, 136] (host pre-pads; halo = R rows/cols each side), both
    pre-converted to bf16 (the 2e-2 absmax-relative gate leaves ~4x margin
    over bf16's ~5e-3).
  - Device: for each 8x16 pixel block of the shard, one bf16 TensorE matmul
    lhsT = src block [K=C=128, M=128 pixels], rhs = tgt window
    [K=128, N=16x24=384] -> PSUM Gram [128, 384] fp32. Four blocks of Gram
    live in one 4-bank PSUM tile; two strided band-select copies per group
    (DVE lower partitions, ACT upper) convert to fp16 in SBUF, then one DMA
    per group dumps to DRAM.
  - Host: zero-FLOP banded-diagonal gather from the Gram blocks into the
    [B, 81, H, W] output (the 81 needed entries per pixel live at
    n = (mi+di)*24 + (mj+dj), a per-partition-skewed pattern that engine
    access patterns cannot express on-chip).
"""

import numpy as np

R = 4
D = 2 * R + 1          # 9
B, C, H, W = 4, 128, 64, 128
NCORES = 8
HS = H // 2            # 32 rows per core shard
TH = HS + 2 * R        # 40 padded tgt rows per shard
TW = W + 2 * R         # 136 padded tgt cols
BI, BJ = 8, 16         # pixel block: 8 rows x 16 cols = 128 = M
NBI, NBJ = HS // BI, W // BJ   # 4 x 8 = 32 blocks per core
WIN_I, WIN_J = BI + 2 * R, BJ + 2 * R  # 16 x 24 window
NW = WIN_I * WIN_J     # 384 streamed columns per block
NBLK = NBI * NBJ
GRP = 4                # blocks per PSUM group / output DMA group
NGRP = NBLK // GRP     # 8 output DMAs
# Band dump: partition group mi only needs Gram cols 24*mi..24*mi+215, so
# partitions 0..63 (groups 0-3) keep cols 0..287 and partitions 64..127
# (groups 4-7) keep cols 96..383. The PSUM->SBUF copies select the band
# (engines accept partition subranges); the output DMA stays a plain
# full-partition transfer. 25% fewer dump bytes, numerically exact.
BANDW = NW - 4 * WIN_J  # 288
BANDO = 4 * WIN_J       # 96, column offset of the upper-half band
PSLOT = 512            # fp32 cols per PSUM bank; one Gram per bank

_compiled = None


def _build_bass():
    import concourse.mybir as mybir
    from concourse import bacc
    from concourse.tile import TileContext
    from concourse.tile_rust import add_dep_helper

    f32 = mybir.dt.float32
    bf16 = mybir.dt.bfloat16
    f16 = mybir.dt.float16
    nc = bacc.Bacc()
    # single combined input: [C, HS*W (block-reordered src) ++ TH*TW (padded
    # tgt)], bf16.
    E = HS * W + TH * TW
    inp = nc.dram_tensor("inp", [C, E], bf16, kind="ExternalInput")
    # Gram band dump, group-major: NGRP groups of GRP blocks staged
    # contiguously so each group leaves as one plain [128, GRP*BANDW] DMA.
    gout = nc.dram_tensor("gout", [NGRP, 128, GRP * BANDW], f16,
                          kind="ExternalOutput")
    gout_ap = gout.ap()

    with TileContext(nc) as tc:
        with (
            tc.tile_pool(name="inp", bufs=1) as inp_pool,
            tc.tile_pool(name="g", bufs=NGRP) as gpool,
            tc.tile_pool(name="psum", bufs=2, space="PSUM") as psum_pool,
        ):
            # src arrives block-reordered from the host: [C, blk, 128 pixels]
            # so each block's weights are one contiguous free dim (the matmul
            # stationary operand allows only one free dimension).
            a = inp_pool.tile([C, E], bf16)
            s = a[:, :HS * W]
            t = a[:, HS * W:].rearrange("c (i j) -> c i j", j=TW)

            # PE warm-up: dummy matmuls during the input-DMA wait start the
            # PE power ramp before the real matmuls arrive. They accumulate
            # into psum buf0, which the tile framework serializes before the
            # first real matmul group that reuses it.
            warm = inp_pool.tile([128, PSLOT], bf16)
            nc.vector.memset(warm, 0.0)
            wtile = psum_pool.tile([128, GRP * PSLOT], f32, tag="ps")
            for _ in range(8):
                nc.tensor.matmul(wtile[:1, :PSLOT], warm[:, :1], warm,
                                 start=True, stop=True)

            # Chunked input load. The SDMA engines round-robin across queues,
            # so concurrent chunk DMAs all finish together; chain the
            # non-critical chunks behind the three the first block-row needs
            # so those get full bandwidth.
            SRCC = NBJ * 128            # src chunk: one block-row = 1024 elems
            TGTC = 8 * TW               # tgt chunk: 8 rows
            def src_chunk(i):
                return nc.sync.dma_start(
                    out=a[:, i * SRCC:(i + 1) * SRCC],
                    in_=inp.ap()[:, i * SRCC:(i + 1) * SRCC])
            def tgt_chunk(i):
                o = HS * W + i * TGTC
                return nc.sync.dma_start(out=a[:, o:o + TGTC],
                                         in_=inp.ap()[:, o:o + TGTC])
            first = [src_chunk(0), tgt_chunk(0), tgt_chunk(1),
                     src_chunk(1), tgt_chunk(2)]
            rest = [src_chunk(2), tgt_chunk(3),
                    src_chunk(3), tgt_chunk(4)]
            for r in rest:
                for f in first:
                    add_dep_helper(r.ins, f.ins,
                                   reason="critical input chunks drain first")

            for grp in range(NGRP):
                stage = gpool.tile([128, GRP * BANDW], f16)
                ps = psum_pool.tile([128, GRP * PSLOT], f32, tag="ps")
                for k in range(GRP):
                    blk = grp * GRP + k
                    bi, bj = divmod(blk, NBJ)
                    lhsT = s[:, blk * 128:(blk + 1) * 128]
                    rhs = t[:, bi * BI: bi * BI + WIN_I,
                            bj * BJ: bj * BJ + WIN_J]
                    nc.tensor.matmul(ps[:, k * PSLOT: k * PSLOT + NW],
                                     lhsT, rhs, start=True, stop=True)
                # band-select copies, batched over the GRP Grams via strided
                # APs: lower partition half on DVE, upper half (shifted by
                # BANDO) on ACT — balanced in parallel, fp32->fp16 on the fly.
                pv = ps.rearrange("p (g c) -> p g c", c=PSLOT)
                sv = stage.rearrange("p (g c) -> p g c", c=BANDW)
                nc.vector.tensor_copy(sv[0:64], pv[0:64, :, 0:BANDW])
                nc.scalar.copy(sv[64:128], pv[64:128, :, BANDO:BANDO + BANDW])
                nc.sync.dma_start(out=gout_ap[grp], in_=stage)
    nc.finalize()
    return nc


def _get_compiled():
    global _compiled
    if _compiled is None:
        _compiled = _build_bass()
    return _compiled


def _shard_inputs(src, tgt):
    """Build per-core input maps (host-side shard + zero-pad + bf16)."""
    import ml_dtypes

    bf16 = ml_dtypes.bfloat16
    in_maps = []
    for c in range(NCORES):
        b = c // 2
        r0 = HS * (c % 2)
        # block-reorder: [C, NBI, BI, NBJ, BJ] -> [C, (NBI NBJ), (BI BJ)]
        s = np.ascontiguousarray(
            src[b, :, r0:r0 + HS, :]
            .reshape(C, NBI, BI, NBJ, BJ)
            .transpose(0, 1, 3, 2, 4)
        ).reshape(C, HS * W)
        tp = np.zeros((C, TH, TW), dtype=np.float32)
        # The reference's window for output pixel (i, j) covers tgt rows
        # i-2R..i and cols j-2R..j (off-center, faithful to the torch quirk:
        # window indices index the PADDED tensor directly, so padded index
        # i-R+di = tgt row i-2R+di). Device pairs src local row il with
        # shard-padded row il+di, so shard row q holds tgt row r0+q-2R;
        # shard col x holds tgt col x-2R.
        lo = r0 - 2 * R
        hi = r0 + HS
        clo = max(lo, 0)
        tp[:, clo - lo: clo - lo + (hi - clo), 2 * R: 2 * R + W] = tgt[b, :, clo:hi, :]
        inp = np.concatenate([s, tp.reshape(C, TH * TW)], axis=1)
        in_maps.append({"inp": np.ascontiguousarray(inp.astype(bf16))})
    return in_maps


# host-side gather indices: out[k=(di,dj)] at pixel (mi,mj) of a block sits at
# Gram column n = (mi+di)*WIN_J + (mj+dj); the device band dump stores
# columns shifted by BANDO for partition groups mi >= 4.
_mi = np.arange(BI)[:, None, None, None]
_mj = np.arange(BJ)[None, :, None, None]
_di = np.arange(D)[None, None, :, None]
_dj = np.arange(D)[None, None, None, :]
_NIDX = ((_mi + _di) * WIN_J + (_mj + _dj)
         - BANDO * (_mi >= 4)).reshape(BI, BJ, D * D)  # [8,16,81]


def _unshard_output(results):
    out = np.empty((B, D * D, H, W), dtype=np.float32)
    for c in range(NCORES):
        b = c // 2
        r0 = HS * (c % 2)
        g = (results[c]["gout"]
             .astype(np.float32)
             .reshape(NGRP, 128, GRP, BANDW)
             .transpose(0, 2, 1, 3)
             .reshape(NBI, NBJ, BI, BJ, BANDW))
        # gather: v[bi,bj,mi,mj,k] = g[bi,bj,mi,mj,_NIDX[mi,mj,k]]
        v = np.take_along_axis(g, _NIDX[None, None], axis=-1)
        # -> out[b, k, r0+bi*8+mi, bj*16+mj]
        v = v.transpose(4, 0, 2, 1, 3)  # [81, NBI, BI, NBJ, BJ]
        out[b, :, r0:r0 + HS, :] = v.reshape(D * D, HS, W)
    return out


def kernel(src, tgt):
    from concourse.bass_utils import run_bass_kernel_spmd

    src = np.asarray(src, dtype=np.float32)
    tgt = np.asarray(tgt, dtype=np.float32)
    nc = _get_compiled()
    in_maps = _shard_inputs(src, tgt)
    res = run_bass_kernel_spmd(nc, in_maps, core_ids=list(range(NCORES)))
    return _unshard_output(res.results)
